# revision 1
# baseline (speedup 1.0000x reference)
"""AttnBlock Trainium2 kernel: B=8 data-parallel across 8 NeuronCores."""
import numpy as np
from contextlib import ExitStack

import concourse.bass as bass
import concourse.tile as tile
from concourse import mybir
from concourse.masks import make_identity

F32 = mybir.dt.float32
F32R = mybir.dt.float32r
BF16 = mybir.dt.bfloat16
AF = mybir.ActivationFunctionType
OP = mybir.AluOpType

C = 64
HW = 4096
EPS = 1e-6
NGROUPS = 32
SCALE = 0.125  # C ** -0.5

# exp grouping per 8-n-block chunk (PSUM 3-bank groups)
CHUNK_GROUPS = [3, 3, 2]


def r(ap):
    return ap.bitcast(F32R)


def split_excess_waits(nc, limit=1):
    """walrus (CoreV3 gen3) rejects >1 sync wait on an instruction; move
    excess waits onto same-engine nops inserted immediately before."""
    nfix = 0
    for bb in nc.main_func.blocks:
        il = bb.instructions
        idx = 0
        while idx < len(il):
            ins = il[idx]
            si = ins.sync_info
            if si is not None and len(si.on_wait) > limit:
                waits = list(si.on_wait)
                keep, extra = waits[:limit], waits[limit:]
                eng = ins.engine
                pos = idx
                while extra:
                    chunk, extra = extra[:limit], extra[limit:]
                    nop_ins = nc.engines[eng].nop(nofuse=True).ins
                    for b2 in nc.main_func.blocks:
                        l2 = b2.instructions
                        for j in range(len(l2) - 1, -1, -1):
                            if l2[j].name == nop_ins.name:
                                del l2[j]
                                break
                    nop_ins.sync_info = mybir.SyncInfo(on_wait=chunk, on_update=[])
                    il.insert(pos, nop_ins)
                    pos += 1
                    idx += 1
                ins.sync_info = mybir.SyncInfo(on_wait=keep, on_update=list(si.on_update))
                nfix += 1
            idx += 1
    return nfix


def build():
    nc = bass.Bass()

    xb = nc.declare_dram_parameter("x", [HW, C], F32, isOutput=False)
    ns_row = nc.declare_dram_parameter("norm_scale", [1, C], F32, isOutput=False)
    wq = nc.declare_dram_parameter("wq", [C, C], F32, isOutput=False)
    bq = nc.declare_dram_parameter("bq", [C, 1], F32, isOutput=False)
    wk = nc.declare_dram_parameter("wk", [C, C], F32, isOutput=False)
    bk = nc.declare_dram_parameter("bk", [C, 1], F32, isOutput=False)
    wv = nc.declare_dram_parameter("wv", [C, C], F32, isOutput=False)
    bv = nc.declare_dram_parameter("bv", [C, 1], F32, isOutput=False)
    wp = nc.declare_dram_parameter("wp", [C, C], F32, isOutput=False)
    bp_row = nc.declare_dram_parameter("bp", [1, C], F32, isOutput=False)
    out = nc.declare_dram_parameter("out", [HW, C], F32, isOutput=True)

    with tile.TileContext(nc) as tc, ExitStack() as S:
        const = S.enter_context(tc.tile_pool(name="const", bufs=1))
        persist = S.enter_context(tc.tile_pool(name="persist", bufs=1))
        dram = S.enter_context(tc.tile_pool(name="dram", bufs=1, space="DRAM"))

        # ---- constants ----
        ident = const.tile([128, 128], F32)
        make_identity(nc, ident)
        identb = const.tile([128, 128], BF16)
        nc.vector.tensor_copy(out=identb, in_=ident)
        ones_col = const.tile([128, 1], F32)
        nc.vector.memset(ones_col, 1.0)
        ones_row = const.tile([1, C], F32)
        nc.vector.memset(ones_row, 1.0)
        one_1 = const.tile([1, 1], F32)
        nc.vector.memset(one_1, 1.0)
        eps_sb = const.tile([1, 1], F32)
        nc.vector.memset(eps_sb, EPS)

        # ---- weight/bias loads ----
        wq_sb = const.tile([C, C], F32)
        wk_sb = const.tile([C, C], F32)
        wv_sb = const.tile([C, C], F32)
        nc.sync.dma_start(out=wq_sb, in_=wq[:, :])
        nc.sync.dma_start(out=wk_sb, in_=wk[:, :])
        nc.sync.dma_start(out=wv_sb, in_=wv[:, :])
        wp_stack = const.tile([128, C], F32)
        wpb = const.tile([128, C], BF16)
        nc.sync.dma_start(out=wp_stack[0:64, :], in_=wp[:, :])
        nc.sync.dma_start(out=wp_stack[64:128, :], in_=wp[:, :])
        nc.vector.tensor_copy(out=wpb, in_=wp_stack)
        bq_col = const.tile([C, 1], F32)
        bk_col = const.tile([C, 1], F32)
        bv_col = const.tile([C, 1], F32)
        nc.sync.dma_start(out=bq_col, in_=bq[:, :])
        nc.sync.dma_start(out=bk_col, in_=bk[:, :])
        nc.sync.dma_start(out=bv_col, in_=bv[:, :])
        bp_r = const.tile([1, C], F32)
        nc.sync.dma_start(out=bp_r, in_=bp_row[:, :])
        ns_r = const.tile([1, C], F32)
        nc.sync.dma_start(out=ns_r, in_=ns_row[:, :])

        # ---- persistent tiles (attention operands in bf16) ----
        Xres = persist.tile([C, C, C], F32)   # [i, w, s] residual
        Qm = [persist.tile([128, 1024], BF16, tag=f"qm{m}", name=f"Qm{m}") for m in range(4)]
        Kc = [persist.tile([128, 16, C], BF16, tag=f"kc{j}", name=f"Kc{j}") for j in range(4)]
        Vpc = [persist.tile([128, 8, 65], BF16, tag=f"vpc{j}", name=f"Vpc{j}") for j in range(4)]
        for j in range(4):
            nc.vector.memset(Vpc[j].rearrange("p n e -> p (n e)")[:, 64::65], 1.0)
        bp_sb = const.tile([C, C], F32)
        out_sb = [persist.tile([C, 16, C], F32, tag=f"osb{j}", name=f"OutSb{j}") for j in range(4)]

        # DRAM scratch (bf16)
        qsc = dram.tile([HW, C], BF16)
        ksc = dram.tile([C, HW], BF16)
        vsc = dram.tile([C, HW], BF16)

        with nc.named_scope("prologue"), \
             tc.tile_pool(name="ph0", bufs=1) as ph0, \
             tc.tile_pool(name="ph0s", bufs=1) as ph0s:
            Xs = ph0.tile([128, 32, C], F32)  # [p, t, c]; row hw = t*128+p
            nc.sync.dma_start(out=Xs, in_=xb.rearrange("(t p) c -> p t c", p=128))
            Xsb = ph0.tile([128, 32, C], BF16)
            nc.vector.tensor_copy(out=Xsb, in_=Xs)
            XT = ph0.tile([C, HW], BF16)      # raw x^T [c, hw] (bf16)
            with tc.tile_pool(name="ps0", bufs=1, space="PSUM") as ps0, \
                 tc.tile_pool(name="ps0t", bufs=3, space="PSUM") as ps0t:
                # --- X^T transposes (pairs share one psum tile) ---
                for t2 in range(16):
                    xt_ps = ps0t.tile([C, 2, 128], BF16, tag="xt")
                    nc.tensor.transpose(xt_ps[:, 0, :], Xsb[:, 2 * t2, :], identb)
                    nc.tensor.transpose(xt_ps[:, 1, :], Xsb[:, 2 * t2 + 1, :], identb)
                    if t2 % 2 == 0:
                        nc.vector.tensor_copy(out=XT[:, t2 * 256:(t2 + 1) * 256],
                                              in_=xt_ps.rearrange("c a b -> c (a b)"))
                    else:
                        nc.scalar.copy(out=XT[:, t2 * 256:(t2 + 1) * 256],
                                       in_=xt_ps.rearrange("c a b -> c (a b)"))
                nc.sync.dma_start(out=Xres, in_=xb.rearrange("(i w) s -> i w s", w=C))
                # --- stats ---
                XSQ = ph0.tile([128, 32, C], BF16)
                nc.vector.tensor_tensor(out=XSQ, in0=Xsb, in1=Xsb, op=OP.mult)
                ones_colb = const.tile([128, 1], BF16)
                nc.vector.memset(ones_colb, 1.0)
                sum_ps = ps0.tile([1, 2, C], F32, tag="sums")
                for t in range(32):
                    nc.tensor.matmul(sum_ps[:, 0, :], ones_colb, Xsb[:, t, :],
                                     start=(t == 0), stop=(t == 31))
                for t in range(32):
                    nc.tensor.matmul(sum_ps[:, 1, :], ones_colb, XSQ[:, t, :],
                                     start=(t == 0), stop=(t == 31))
                sums_sb = ph0s.tile([1, 2, C], F32)
                nc.vector.tensor_copy(out=sums_sb, in_=sum_ps)
                sx_pair = sums_sb[:, 0, :].rearrange("o (g t) -> o g t", t=2)
                sq_pair = sums_sb[:, 1, :].rearrange("o (g t) -> o g t", t=2)
                mu = ph0s.tile([1, NGROUPS], F32)
                ex2 = ph0s.tile([1, NGROUPS], F32)
                nc.vector.tensor_tensor(out=mu, in0=sx_pair[:, :, 0], in1=sx_pair[:, :, 1], op=OP.add)
                nc.vector.tensor_tensor(out=ex2, in0=sq_pair[:, :, 0], in1=sq_pair[:, :, 1], op=OP.add)
                nc.vector.tensor_scalar(out=mu, in0=mu, scalar1=1.0 / 8192.0, scalar2=None, op0=OP.mult)
                nc.vector.tensor_scalar(out=ex2, in0=ex2, scalar1=1.0 / 8192.0, scalar2=None, op0=OP.mult)
                musq = ph0s.tile([1, NGROUPS], F32)
                nc.vector.tensor_tensor(out=musq, in0=mu, in1=mu, op=OP.mult)
                var = ph0s.tile([1, NGROUPS], F32)
                nc.vector.tensor_tensor(out=var, in0=ex2, in1=musq, op=OP.subtract)
                sd = ph0s.tile([1, NGROUPS], F32)
                nc.scalar.activation(out=sd, in_=var, func=AF.Sqrt, bias=eps_sb[:, :])
                rstd = ph0s.tile([1, NGROUPS], F32)
                nc.vector.reciprocal(out=rstd, in_=sd)
                # prefetch the exp table set during prologue idle time
                expwarm = ph0s.tile([1, 1], F32)
                nc.scalar.activation(out=expwarm, in_=rstd[:, 0:1], func=AF.Exp, scale=0.0)
                Atmp = ph0s.tile([1, C], F32)
                Btmp = ph0s.tile([1, C], F32)
                A_row = ph0s.tile([1, C], F32)
                B_row = ph0s.tile([1, C], F32)
                ap2 = Atmp.rearrange("o (g t) -> o g t", t=2)
                bp2 = Btmp.rearrange("o (g t) -> o g t", t=2)
                nc.vector.tensor_copy(out=ap2[:, :, 0], in_=rstd)
                nc.vector.tensor_copy(out=ap2[:, :, 1], in_=rstd)
                nc.vector.tensor_copy(out=bp2[:, :, 0], in_=mu)
                nc.vector.tensor_copy(out=bp2[:, :, 1], in_=mu)
                nc.vector.tensor_tensor(out=A_row, in0=Atmp, in1=ns_r, op=OP.mult)
                Btmp2 = ph0s.tile([1, C], F32)
                nc.vector.tensor_tensor(out=Btmp2, in0=Btmp, in1=A_row, op=OP.mult)
                nc.vector.tensor_scalar(out=B_row, in0=Btmp2, scalar1=-1.0, scalar2=None, op0=OP.mult)
                # rows -> columns
                col_ps = ps0.tile([C, 2, 1], F32, tag="cols")
                nc.tensor.matmul(col_ps[:, 0, :], A_row, one_1, start=True, stop=True)
                nc.tensor.matmul(col_ps[:, 1, :], B_row, one_1, start=True, stop=True)
                A_col = ph0s.tile([C, 1], F32)
                B_col = ph0s.tile([C, 1], F32)
                nc.vector.tensor_copy(out=A_col, in_=col_ps[:, 0, :])
                nc.vector.tensor_copy(out=B_col, in_=col_ps[:, 1, :])
                # bp broadcast [64, 64]
                bp_ps = ps0.tile([C, C], F32, tag="bpb")
                nc.tensor.matmul(bp_ps, ones_row, bp_r, start=True, stop=True)
                nc.vector.tensor_copy(out=bp_sb, in_=bp_ps)
                # fold groupnorm affine into qkv weights/biases
                wqs = ph0s.tile([C, C], BF16)
                wks = ph0s.tile([C, C], BF16)
                wvs = ph0s.tile([C, C], BF16)
                bq2 = ph0s.tile([C, 1], F32)
                bk2 = ph0s.tile([C, 1], F32)
                bv2 = ph0s.tile([C, 1], F32)
                bias_ps = ps0.tile([C, 3, 1], F32, tag="bias")
                for i, (w_sb, b_col, ws, b2) in enumerate((
                        (wq_sb, bq_col, wqs, bq2), (wk_sb, bk_col, wks, bk2),
                        (wv_sb, bv_col, wvs, bv2))):
                    nc.vector.tensor_scalar(out=ws, in0=w_sb, scalar1=A_col,
                                            scalar2=None, op0=OP.mult)
                    nc.tensor.matmul(bias_ps[:, i, :], w_sb, B_col, start=True, stop=True)
                    nc.vector.tensor_tensor(out=b2, in0=bias_ps[:, i, :], in1=b_col, op=OP.add)

            # ---- QKV (q first: its chain gates attention) ----
            with nc.named_scope("qkv"), \
                 tc.tile_pool(name="ph2", bufs=1) as ph2:
                qT = ph2.tile([C, HW], BF16)
                kT = ph2.tile([C, HW], BF16)
                vT = ph2.tile([C, HW], BF16)
                with tc.tile_pool(name="ps2", bufs=4, space="PSUM") as ps2, \
                     tc.tile_pool(name="ph3", bufs=1) as ph3, \
                     tc.tile_pool(name="ps3", bufs=3, space="PSUM") as ps3:
                    for ti, (ws, b2, dst) in enumerate(((wqs, bq2, qT), (wks, bk2, kT), (wvs, bv2, vT))):
                        for j in range(8):
                            mm_ps = ps2.tile([C, 512], F32, tag="qkv")
                            nc.tensor.matmul(mm_ps, ws, XT[:, j * 512:(j + 1) * 512],
                                             start=True, stop=True)
                            if j % 2 == 0:
                                nc.vector.tensor_scalar(out=dst[:, j * 512:(j + 1) * 512],
                                                        in0=mm_ps, scalar1=b2, scalar2=None,
                                                        op0=OP.add)
                            else:
                                nc.scalar.activation(out=dst[:, j * 512:(j + 1) * 512],
                                                     in_=mm_ps, func=AF.Identity,
                                                     bias=b2, scale=1.0)
                        if ti == 0:
                            # q -> natural layout via PE transpose (bf16), batched store
                            qn = ph3.tile([128, 32, C], BF16)
                            for t2 in range(16):
                                qn_ps = ps3.tile([128, 2, C], BF16, tag="qn")
                                nc.tensor.transpose(qn_ps[:, 0, :], qT[:, 2 * t2 * 128:(2 * t2 + 1) * 128],
                                                    identb[0:64, 0:64])
                                nc.tensor.transpose(qn_ps[:, 1, :], qT[:, (2 * t2 + 1) * 128:(2 * t2 + 2) * 128],
                                                    identb[0:64, 0:64])
                                if t2 % 2 == 0:
                                    nc.vector.tensor_copy(out=qn[:, 2 * t2:2 * t2 + 2, :], in_=qn_ps)
                                else:
                                    nc.scalar.copy(out=qn[:, 2 * t2:2 * t2 + 2, :], in_=qn_ps)
                            nc.sync.dma_start(out=qsc.rearrange("(t p) c -> p t c", p=128), in_=qn)
                            qv = qsc.rearrange("(h w) d -> h (w d)", h=C)
                            nc.sync.dma_start(out=Qm[0][0:64, :], in_=qv[:, 0:1024])
                            nc.sync.dma_start(out=Qm[0][64:128, :], in_=qv[:, 0:1024])
                        elif ti == 1:
                            nc.sync.dma_start(out=ksc[0:16, :], in_=kT[0:16, :])
                            nc.sync.dma_start(out=ksc[16:64, :], in_=kT[16:64, :])
                            kv = ksc.rearrange("d (h w) -> h d w", h=C)
                            nc.sync.dma_start(out=Kc[0][0:64, :, :], in_=kv[:, 0:16, :])
                            nc.sync.dma_start(out=Kc[0][64:128, :, :], in_=kv[:, 0:16, :])
                        else:
                            nc.sync.dma_start(out=vsc[:, :], in_=vT)

        # ---- V' chunks: load + transpose ----
        with nc.named_scope("vload"), \
             tc.tile_pool(name="ph4", bufs=2) as ph4, \
             tc.tile_pool(name="ps4", bufs=3, space="PSUM") as ps4:
            vv = vsc.rearrange("d (h w) -> h d w", h=C)
            qv2 = qsc.rearrange("(h w) d -> h (w d)", h=C)
            kv2 = ksc.rearrange("d (h w) -> h d w", h=C)
            for j in range(4):
                vchunk = ph4.tile([C, 16, C], BF16, tag="vchunk")
                nc.sync.dma_start(out=vchunk, in_=vv[:, j * 16:(j + 1) * 16, :])
                if j >= 1:
                    nc.sync.dma_start(out=Kc[j][0:64, :, :], in_=kv2[:, j * 16:(j + 1) * 16, :])
                    nc.sync.dma_start(out=Kc[j][64:128, :, :], in_=kv2[:, j * 16:(j + 1) * 16, :])
                    nc.sync.dma_start(out=Qm[j][0:64, :], in_=qv2[:, j * 1024:(j + 1) * 1024])
                    nc.sync.dma_start(out=Qm[j][64:128, :], in_=qv2[:, j * 1024:(j + 1) * 1024])
                for b2i in range(4):
                    vt_ps = ps4.tile([128, 2, C], BF16, tag="vt")
                    nc.tensor.transpose(vt_ps[:, 0, :],
                                        vchunk[:, 4 * b2i:4 * b2i + 2, :].rearrange("h d w -> h (d w)"),
                                        identb[0:64, 0:64])
                    nc.tensor.transpose(vt_ps[:, 1, :],
                                        vchunk[:, 4 * b2i + 2:4 * b2i + 4, :].rearrange("h d w -> h (d w)"),
                                        identb[0:64, 0:64])
                    dst = Vpc[j].rearrange("p n e -> p n e")[:, 2 * b2i:2 * b2i + 2, 0:64]
                    if b2i % 2 == 0:
                        nc.vector.tensor_copy(out=dst, in_=vt_ps)
                    else:
                        nc.scalar.copy(out=dst, in_=vt_ps)

        # ---- attention over m-tiles ----
        with nc.named_scope("attn"), \
             tc.tile_pool(name="sexp", bufs=2) as sexp, \
             tc.tile_pool(name="mrow", bufs=3) as mrow, \
             tc.tile_pool(name="psS", bufs=2, space="PSUM") as psS, \
             tc.tile_pool(name="psO", bufs=1, space="PSUM") as psO, \
             tc.tile_pool(name="psE", bufs=1, space="PSUM") as psE:

            def epilogue_rest(mt, rsrow, Osb):
                epool, etag = (psS, "sgrp") if mt == 7 else (psE, "epi")
                rsT_ps = epool.tile([128, 4], F32, tag=etag, name=f"rsT{mt}")
                for j in range(4):
                    nc.tensor.transpose(rsT_ps[:, j:j + 1], rsrow[:, j * 128:(j + 1) * 128], one_1)
                rcol = mrow.tile([128, 4], F32, tag="rcol", name=f"rcol{mt}")
                nc.vector.tensor_copy(out=rcol, in_=rsT_ps)
                nc.vector.reciprocal(out=rcol, in_=rcol)
                for j in range(4):
                    ot_ps = epool.tile([128, C], F32, tag=etag, name=f"ot{mt}_{j}")
                    nc.tensor.transpose(ot_ps, Osb[:, j * 128:(j + 1) * 128], ident[0:64, 0:64])
                    OTn = mrow.tile([128, C], BF16, tag="otn", name=f"otn{mt}_{j}")
                    nc.vector.tensor_scalar(out=OTn, in0=ot_ps, scalar1=rcol[:, j:j + 1],
                                            scalar2=None, op0=OP.mult)
                    for half in range(2):
                        w_idx = mt * 8 + 2 * j + half
                        fin_ps = epool.tile([C, C], F32, tag=etag, name=f"fin{mt}_{j}_{half}")
                        nc.tensor.matmul(fin_ps, OTn[64 * half:64 * half + 64, :],
                                         wpb[64 * half:64 * half + 64, :],
                                         start=True, stop=True)
                        ou1 = mrow.tile([C, C], F32, tag="ou1", name=f"ou1_{mt}_{j}_{half}")
                        nc.vector.tensor_tensor(out=ou1, in0=fin_ps, in1=bp_sb, op=OP.add)
                        nc.vector.tensor_tensor(out=out_sb[w_idx // 16][:, w_idx % 16, :],
                                                in0=ou1, in1=Xres[:, w_idx, :], op=OP.add)

            GRP = [3, 3, 3, 3, 3, 3, 3, 3, 3, 3, 2]
            sched = []
            for mt in range(8):
                nb = 0
                for gi, g_sz in enumerate(GRP):
                    sched.append((mt, gi, nb, g_sz))
                    nb += g_sz

            sg_tiles = {}

            def emit_S(idx):
                mt, gi, nb, g_sz = sched[idx]
                sg = psS.tile([128, 3, 512], F32, tag="sgrp", name=f"sg{idx}")
                sg_tiles[idx] = sg
                mo = (mt % 2) * 512
                k = 0
                while k < g_sz:
                    kk = nb + k
                    if k + 1 < g_sz and idx > 1:
                        kk2 = kk + 1
                        nc.tensor.matmul(
                            sg[:, k, :],
                            Kc[kk // 8][0:64, 2 * (kk % 8):2 * (kk % 8) + 2, :].rearrange("h d w -> h (d w)"),
                            Qm[mt // 2][0:64, mo:mo + 512],
                            start=True, stop=True, tile_position=(0, 0))
                        nc.tensor.matmul(
                            sg[:, k + 1, :],
                            Kc[kk2 // 8][64:128, 2 * (kk2 % 8):2 * (kk2 % 8) + 2, :].rearrange("h d w -> h (d w)"),
                            Qm[mt // 2][64:128, mo:mo + 512],
                            start=True, stop=True, tile_position=(64, 0))
                        k += 2
                    else:
                        nc.tensor.matmul(
                            sg[:, k, :],
                            Kc[kk // 8][0:64, 2 * (kk % 8):2 * (kk % 8) + 2, :].rearrange("h d w -> h (d w)"),
                            Qm[mt // 2][0:64, mo:mo + 512],
                            start=True, stop=True, tile_position=(0, 0))
                        k += 1

            pending = None
            oacc = None
            emit_S(0)
            for idx, (mt, gi, nb, g_sz) in enumerate(sched):
                if gi == 0:
                    oacc = psO.tile([65, 512], F32, tag="oacc", name=f"oacc{mt}")
                sg = sg_tiles.pop(idx)
                eA = sexp.tile([128, 3, 512], BF16, tag="ea", bufs=6)
                nc.scalar.activation(out=eA.rearrange("p a b -> p (a b)")[:, 0:g_sz * 512],
                                     in_=sg.rearrange("p a b -> p (a b)")[:, 0:g_sz * 512],
                                     func=AF.Exp, scale=SCALE)
                if idx + 1 < len(sched):
                    emit_S(idx + 1)
                for k in range(g_sz):
                    kk = nb + k
                    nc.tensor.matmul(oacc, Vpc[kk // 8][:, kk % 8, :],
                                     eA[:, k, :],
                                     start=(kk == 0), stop=(kk == 31))
                if gi == 0 and pending is not None:
                    mt_done = pending[0]
                    epilogue_rest(*pending)
                    pending = None
                    if mt_done in (1, 3, 5):
                        q4 = (mt_done - 1) // 2
                        nc.sync.dma_start(
                            out=out.rearrange("(i w) s -> i w s", w=C)[:, q4 * 16:(q4 + 1) * 16, :],
                            in_=out_sb[q4])
                if gi == len(GRP) - 1:
                    # drain oacc quickly so the next m-tile can reuse the bank
                    rsrow = mrow.tile([1, 512], F32, tag="rsrow", name=f"rsrow{mt}")
                    nc.vector.tensor_copy(out=rsrow, in_=oacc[64:65, :])
                    Osb = mrow.tile([C, 512], F32, tag="osb2", name=f"Osb{mt}")
                    nc.vector.tensor_copy(out=Osb, in_=oacc[0:64, :])
                    pending = (mt, rsrow, Osb)
            epilogue_rest(*pending)
            nc.sync.dma_start(out=out.rearrange("(i w) s -> i w s", w=C)[:, 48:64, :],
                              in_=out_sb[3])

    nfix = split_excess_waits(nc)
    return nc, nfix

# ---------------------------------------------------------------------------
# Self-contained entry: kernel(**inputs) -> np.ndarray of shape (8, 64, 64, 64)
# Shards batch B=8 across the 8 NeuronCores (data parallel); each core runs
# the full AttnBlock on its batch element.
# ---------------------------------------------------------------------------
_BUILT = None


def _get_built():
    global _BUILT
    if _BUILT is None:
        nc, _ = build()
        _BUILT = nc
    return _BUILT


def kernel(x, norm_scale, wq, bq, wk, bk, wv, bv, wp, bp):
    from concourse.bass_utils import run_bass_kernel_spmd

    B = 8
    nc = _get_built()
    x = np.ascontiguousarray(np.asarray(x, dtype=np.float32))
    shared = {
        "norm_scale": np.asarray(norm_scale, np.float32).reshape(1, C),
        "wq": np.asarray(wq, np.float32), "bq": np.asarray(bq, np.float32).reshape(C, 1),
        "wk": np.asarray(wk, np.float32), "bk": np.asarray(bk, np.float32).reshape(C, 1),
        "wv": np.asarray(wv, np.float32), "bv": np.asarray(bv, np.float32).reshape(C, 1),
        "wp": np.asarray(wp, np.float32),
        "bp": np.asarray(bp, np.float32).reshape(1, C),
    }
    in_maps = [dict(shared, x=x[b].reshape(HW, C)) for b in range(B)]
    res = run_bass_kernel_spmd(nc, in_maps, list(range(B)))
    out = np.stack([res.results[b]["out"].reshape(C, C, C) for b in range(B)])
    return out.astype(np.float32)

